# revision 41
# baseline (speedup 1.0000x reference)
"""Trainium2 Bass kernel for nn_KnowledgeAttention.

Math (per batch example b):
    sim[k]  = cos_sim(pooled[b], kg_key[b,k])                      # [K]
    q       = (hs @ Wq.T + bq) * HD**-0.5     -> heads [T,H,HD]
    k       = kg_value @ Wk.T + bk            -> heads [K,H,HD]
    v       = kg_value @ Wv.T + bv            -> heads [K,H,HD]
    S[h,t,k]= q_h[t]·k_h[k] + beta[h]*sim[k]
    P       = softmax_k(S);  O[t,h] = sum_k P v
    out     = O @ Wo.T + bo

Sharding: pure data-parallel over batch — 8 examples on 8 cores, weights
replicated, no collectives.

Per-core layout strategy (all matmul contractions run on the partition dim):
    hs.T, kg_value.T via PE transpose; q.T/k.T/v from projections;
    scores computed transposed S.T[k,t] so the cosine-sim bias is a
    per-partition scalar folded into the ACT exp bias; attention output
    O.T[d,t] feeds the final projection lhsT directly; softmax denominators
    via ones-matmuls; normalization uses a gpsimd partition-broadcast of the
    reciprocal row. Matmuls in bf16 with fp32 PSUM accumulation.
"""

import sys

import numpy as np

# ---------------------------------------------------------------- constants
BS = 8
T = 2048
D = 768
H = 12
HD = 64
K = 512
SCALE = HD ** -0.5
EPS = 1e-8
DC = D // 128   # 6 contraction/partition chunks of 128 over D
KC = K // 128   # 4 chunks over K
TW = 512        # t window for moving operand
NTW = T // TW   # 4
NPAIR = H // 2  # 6 head pairs

TRACE = False
LAST_EXEC_NS = None

_CACHE = {}


def _ensure_path():
    try:
        import concourse  # noqa: F401
    except ImportError:
        for p in ("/opt/trn_rl_repo", "/root/.axon_site/_ro/trn_rl_repo"):
            if p not in sys.path:
                sys.path.insert(0, p)


def _build_program():
    _ensure_path()
    import concourse.bass as bass
    import concourse.mybir as mybir
    import concourse.tile as tile
    from concourse import bacc
    from concourse.masks import make_identity
    from contextlib import ExitStack

    F32 = mybir.dt.float32
    BF16 = mybir.dt.bfloat16
    Alu = mybir.AluOpType
    Act = mybir.ActivationFunctionType

    nc = bacc.Bacc("TRN2", target_bir_lowering=False, debug=False, num_devices=BS)

    hs_d = nc.dram_tensor("hs", [T, D], BF16, kind="ExternalInput").ap()
    kgk_d = nc.dram_tensor("kgk", [K, D], BF16, kind="ExternalInput").ap()
    kgv_d = nc.dram_tensor("kgv", [K, D], BF16, kind="ExternalInput").ap()
    pl_d = nc.dram_tensor("pooled", [1, D], BF16, kind="ExternalInput").ap()
    wqt_d = nc.dram_tensor("wqt", [D, D], BF16, kind="ExternalInput").ap()
    wkt_d = nc.dram_tensor("wkt", [D, D], BF16, kind="ExternalInput").ap()
    wvt_d = nc.dram_tensor("wvt", [D, D], BF16, kind="ExternalInput").ap()
    wot_d = nc.dram_tensor("wot", [D, D], BF16, kind="ExternalInput").ap()
    bq_d = nc.dram_tensor("bq", [128, DC], F32, kind="ExternalInput").ap()
    bk_d = nc.dram_tensor("bk", [128, DC], F32, kind="ExternalInput").ap()
    bo_d = nc.dram_tensor("bo", [1, D], F32, kind="ExternalInput").ap()
    beta_d = nc.dram_tensor("beta", [1, H], F32, kind="ExternalInput").ap()
    out_d = nc.dram_tensor("out", [T, D], F32, kind="ExternalOutput").ap()

    with tile.TileContext(nc) as tc, ExitStack() as ctx:
        const = ctx.enter_context(tc.tile_pool(name="const", bufs=1))
        inp = ctx.enter_context(tc.tile_pool(name="inp", bufs=6))
        wpool = ctx.enter_context(tc.tile_pool(name="w", bufs=12))
        big = ctx.enter_context(tc.tile_pool(name="big", bufs=12))
        hstw_p = ctx.enter_context(tc.tile_pool(name="hstw", bufs=12))
        kt_p = ctx.enter_context(tc.tile_pool(name="ktp", bufs=6))
        v_p = ctx.enter_context(tc.tile_pool(name="vp", bufs=4))
        kgt_p = ctx.enter_context(tc.tile_pool(name="kgtp", bufs=6))
        e_p = ctx.enter_context(tc.tile_pool(name="ep", bufs=12))
        r_p = ctx.enter_context(tc.tile_pool(name="rp", bufs=4))
        rb_p = ctx.enter_context(tc.tile_pool(name="rbp", bufs=2))
        fin_p = ctx.enter_context(tc.tile_pool(name="finp", bufs=2))
        sm_p = ctx.enter_context(tc.tile_pool(name="smp", bufs=4))
        ps = ctx.enter_context(tc.tile_pool(name="ps", bufs=2, space="PSUM"))

        # ---------------- phase 0: constants + cosine-sim bias ----------------
        ident = const.tile([128, 128], BF16, tag="ident")
        make_identity(nc, ident[:])
        ones_bf = const.tile([128, 64], BF16, tag="ones_bf")
        nc.vector.memset(ones_bf[:], 1.0)
        # kg_value loads first: transposes are the critical path into phase 1
        kv_tiles = []
        for c in range(KC):
            kv = inp.tile([128, D], BF16, tag="inp", name="kv")
            nc.sync.dma_start(kv[:], kgv_d[c * 128:(c + 1) * 128, :])
            kv_tiles.append(kv)
        wk_sb = []
        for c in range(DC):
            wk = wpool.tile([128, D], BF16, tag="w")
            nc.sync.dma_start(wk[:], wkt_d[c * 128:(c + 1) * 128, :])
            wk_sb.append(wk)
        kk_tiles = []
        for c in range(KC):
            kk = inp.tile([128, D], BF16, tag="kgk", bufs=4, name="kk")
            nc.sync.dma_start(kk[:], kgk_d[c * 128:(c + 1) * 128, :])
            kk_tiles.append(kk)

        pl = const.tile([1, D], BF16, tag="pl")
        nc.sync.dma_start(pl[:], pl_d)
        bt = const.tile([1, H], F32, tag="bt")
        nc.sync.dma_start(bt[:], beta_d)
        bo_row = const.tile([1, D], F32, tag="bo_row")
        nc.sync.dma_start(bo_row[:], bo_d)
        bq_sb = const.tile([128, DC], F32, tag="bq_sb")
        nc.sync.dma_start(bq_sb[:], bq_d)
        bk_sb = const.tile([128, DC], F32, tag="bk_sb")
        nc.sync.dma_start(bk_sb[:], bk_d)

        bo_bc = const.tile([128, D], F32, tag="bo_bc")
        nc.gpsimd.partition_broadcast(bo_bc[:], bo_row[:], channels=128)
        beta_bc = const.tile([128, H], F32, tag="beta_bc")
        nc.gpsimd.partition_broadcast(beta_bc[:], bt[:], channels=128)
        pl_bc = const.tile([128, D], BF16, tag="pl_bc")
        nc.gpsimd.partition_broadcast(pl_bc[:], pl[:], channels=128)

        # pooled 1/||.|| as a per-partition vector (computed on the broadcast)
        pl_sq = inp.tile([128, D], BF16, tag="inp", name="pl_sq")
        pnorm = sm_p.tile([128, 1], F32, tag="pnorm")
        nc.scalar.activation(pl_sq[:], pl_bc[:], Act.Square, accum_out=pnorm[:])
        nc.scalar.activation(pnorm[:], pnorm[:], Act.Sqrt)
        nc.vector.tensor_scalar_max(pnorm[:], pnorm[:], EPS)
        rp_vec = const.tile([128, 1], F32, tag="rp_vec")
        nc.vector.reciprocal(rp_vec[:], pnorm[:])

        # ---------------- phase 1a: kg_value.T, k.T ----------------
        # remaining weight DMAs in consumption order: wq (prep0), wv, wo
        wq_sb = []
        for c in range(DC):
            wq = wpool.tile([128, D], BF16, tag="w")
            nc.sync.dma_start(wq[:], wqt_d[c * 128:(c + 1) * 128, :])
            wq_sb.append(wq)
        hv0_tiles = []
        for tsub in range(TW // 128):
            hv = inp.tile([128, D], BF16, tag="inp", name="hv")
            nc.sync.dma_start(hv[:], hs_d[tsub * 128:(tsub + 1) * 128, :])
            hv0_tiles.append(hv)

        kgt = [kgt_p.tile([128, K], BF16, tag="kgt", name="kgt") for _ in range(DC)]
        for dchunk in range(DC):
            pt = ps.tile([128, K], BF16, tag="mm", bufs=1, name="ptr")
            for c in range(KC):
                nc.tensor.transpose(
                    pt[:, c * 128:(c + 1) * 128],
                    kv_tiles[c][:, dchunk * 128:(dchunk + 1) * 128], ident[:])
            nc.vector.tensor_copy(kgt[dchunk][:], pt[:])

        kt = [kt_p.tile([128, K], BF16, tag="kt", name="kt") for _ in range(DC)]
        for m in range(DC):
            pk = ps.tile([128, K], F32, tag="mm", bufs=1)
            for c in range(DC):
                nc.tensor.matmul(
                    pk[:], wk_sb[c][:, m * 128:(m + 1) * 128], kgt[c][:],
                    start=(c == 0), stop=(c == DC - 1))
            nc.vector.tensor_scalar_add(kt[m][:], pk[:], bk_sb[:, m:m + 1])

        def prep_window(tc4, hv_tiles=None):
            hstw = [hstw_p.tile([128, TW], BF16, tag="hstw", name="hstw")
                    for _ in range(DC)]
            if hv_tiles is None:
                hv_tiles = []
                for tsub in range(TW // 128):
                    hv = inp.tile([128, D], BF16, tag="inp", name="hv")
                    t0 = tc4 * TW + tsub * 128
                    nc.sync.dma_start(hv[:], hs_d[t0:t0 + 128, :])
                    hv_tiles.append(hv)
            for c in range(DC):
                pt = ps.tile([128, TW], BF16, tag="mm", bufs=1, name="ptr")
                for tsub in range(TW // 128):
                    nc.tensor.transpose(
                        pt[:, tsub * 128:(tsub + 1) * 128],
                        hv_tiles[tsub][:, c * 128:(c + 1) * 128], ident[:])
                nc.vector.tensor_copy(hstw[c][:], pt[:])
            qts = [big.tile([128, TW], BF16, tag=f"qt{m}", bufs=2, name="qtw")
                   for m in range(DC)]
            for m in range(DC):
                pq = ps.tile([128, TW], F32, tag="mm", bufs=1)
                for c in range(DC):
                    nc.tensor.matmul(
                        pq[:], wq_sb[c][:, m * 128:(m + 1) * 128], hstw[c][:],
                        start=(c == 0), stop=(c == DC - 1))
                nc.vector.tensor_scalar_add(qts[m][:], pq[:], bq_sb[:, m:m + 1])
            return qts

        qt = prep_window(0, hv0_tiles)

        # ------- cosine-sim bias + ebias (ACT/DVE, overlaps PE prep) -------
        bias_all = const.tile([128, KC * H], F32, tag="bias_all")
        for c in range(KC):
            kk = kk_tiles[c]
            sq = inp.tile([128, D], BF16, tag="inp")
            nrm = sm_p.tile([128, 1], F32, tag="nrm")
            nc.scalar.activation(sq[:], kk[:], Act.Square, accum_out=nrm[:])
            nc.scalar.activation(nrm[:], nrm[:], Act.Sqrt)
            nc.vector.tensor_scalar_max(nrm[:], nrm[:], EPS)
            rn = sm_p.tile([128, 1], F32, tag="rn")
            nc.vector.reciprocal(rn[:], nrm[:])
            sq2 = inp.tile([128, D], BF16, tag="inp")
            dot = sm_p.tile([128, 1], F32, tag="dot")
            nc.vector.scalar_tensor_tensor(
                out=sq2[:], in0=kk[:], scalar=1.0, in1=pl_bc[:],
                op0=Alu.mult, op1=Alu.mult, accum_out=dot[:])
            nc.vector.tensor_mul(dot[:], dot[:], rn[:])
            nc.vector.tensor_mul(dot[:], dot[:], rp_vec[:])
            nc.vector.tensor_scalar_mul(
                bias_all[:, c * H:(c + 1) * H], beta_bc[:], dot[:])

        # ebias[k_part, kc*H + h] = exp(beta[h]*sim[k]) — folded into v and
        # the denominator weights so the softmax exp needs no bias (enables
        # even/odd-merged [128,1024] exps)
        ebias = const.tile([128, KC * H], F32, tag="ebias")
        nc.scalar.activation(ebias[:], bias_all[:], Act.Exp)
        ebias_bf = const.tile([128, KC * H], BF16, tag="ebias_bf")
        nc.vector.tensor_copy(ebias_bf[:], ebias[:])

        def eb_col64(c, h):
            # [128, 64] broadcast view of ebias_bf column kc*H+h (stride-0)
            col = ebias_bf[:, c * H + h:c * H + h + 1].copy()
            col.ap = col.ap[:-1] + [[0, 64]]
            return col

        # scores: even head on PE row-tile (0,0) -> cols 0:512 of a 2-bank
        # tile, odd head on (64,0) -> cols 512:1024; one bias-free exp
        # covers both banks.
        def scores_pair(j, qtw, ts0=0, w=TW):
            # odd head always lands at the TW (bank-aligned) offset; for
            # w < TW one exp covers [0, TW+w) and cols [w, TW) are unused.
            e_j = []
            for kc in range(KC):
                psc = ps.tile([128, TW + w], F32, tag="s", bufs=2)
                nc.tensor.matmul(
                    psc[:, 0:w], kt[j][0:64, kc * 128:(kc + 1) * 128],
                    qtw[j][0:64, ts0:ts0 + w], start=True, stop=True)
                nc.tensor.matmul(
                    psc[:, TW:TW + w],
                    kt[j][64:128, kc * 128:(kc + 1) * 128],
                    qtw[j][64:128, ts0:ts0 + w], start=True, stop=True)
                ebig = e_p.tile([128, 2 * w], BF16, tag="e", bufs=16)
                src = psc[:].copy()
                src.ap = src.ap[:-1] + [[TW, 2], [1, w]]
                dst = ebig[:].copy()
                dst.ap = dst.ap[:-1] + [[w, 2], [1, w]]
                nc.scalar.activation(dst, src, Act.Exp)
                e_j.append(ebig)
            return e_j

        # AV + denominators: even head -> col-tile (0,0) rows 0:64, odd head
        # -> col-tile (0,64) rows 64:128 of the same bank; interleaved issue
        # so both column tiles stream concurrently. pd rows 0:64 = sum_k
        # e_even (64x replicated), rows 64:128 = odd.
        def avden_pair(j, e_j, ot_j, w=TW):
            po = ps.tile([128, w], F32, tag="o", bufs=2, name="po")
            pd = ps.tile([128, w], F32, tag="d", bufs=1, name="pd")
            for kc in range(KC):
                st = (kc == 0)
                sp = (kc == KC - 1)
                nc.tensor.matmul(
                    po[0:64, :],
                    v_sb[kc][:, (2 * j) * HD:(2 * j + 1) * HD],
                    e_j[kc][:, 0:w], start=st, stop=sp)
                nc.tensor.matmul(
                    po[64:128, :],
                    v_sb[kc][:, (2 * j + 1) * HD:(2 * j + 2) * HD],
                    e_j[kc][:, w:2 * w], start=st, stop=sp)
                nc.tensor.matmul(
                    pd[0:64, :], eb_col64(kc, 2 * j),
                    e_j[kc][:, 0:w], start=st, stop=sp)
                nc.tensor.matmul(
                    pd[64:128, :], eb_col64(kc, 2 * j + 1),
                    e_j[kc][:, w:2 * w], start=st, stop=sp)
            rall = r_p.tile([128, w], F32, tag="rall", name="rall")
            # high priority: these free the po/pd banks — keep them ahead
            # of filler DVE work so the next pair's AV/den can start
            with tc.high_priority():
                nc.vector.reciprocal_approx_fast(rall[:], pd[:])
                nc.vector.tensor_mul(ot_j[:], po[:], rall[:])

        def outproj_window(tc16_0, ots, w=TW):
            for tsub in range(w // 128):
                tc16 = tc16_0 + tsub
                fin = fin_p.tile([128, D], F32, tag="fin")
                for n in range(2):
                    pf = ps.tile([128, 384], F32, tag="mm", bufs=1)
                    for c in range(DC):
                        nc.tensor.matmul(
                            pf[:], ots[c][:, tsub * 128:(tsub + 1) * 128],
                            wo_sb[c][:, n * 384:(n + 1) * 384],
                            start=(c == 0), stop=(c == DC - 1))
                    nc.vector.tensor_add(
                        fin[:, n * 384:(n + 1) * 384], pf[:],
                        bo_bc[:, n * 384:(n + 1) * 384])
                nc.sync.dma_start(out_d[tc16 * 128:(tc16 + 1) * 128, :], fin[:])

        # ------- window 0: all scores issued first; the v projection below
        # is the PE filler for the exp-paced stretch; AV/den then consume.
        ots = [big.tile([128, TW], BF16, tag=f"ot{j}", bufs=2, name="otw")
               for j in range(NPAIR)]
        e_w0 = [scores_pair(j, qt) for j in range(NPAIR)]

        # ---------------- v projection (+ ebias fold) ----------------
        wv_sb = []
        for c in range(DC):
            wv = wpool.tile([128, D], BF16, tag="w")
            nc.sync.dma_start(wv[:], wvt_d[c * 128:(c + 1) * 128, :])
            wv_sb.append(wv)
        wo_sb = []
        for c in range(DC):
            wo = wpool.tile([128, D], BF16, tag="w")
            nc.sync.dma_start(wo[:], wot_d[c * 128:(c + 1) * 128, :])
            wo_sb.append(wo)

        v_sb = [v_p.tile([128, D], BF16, tag="v", name="vsb")
                for _ in range(KC)]
        for kc in range(KC):
            for n in range(2):
                pv = ps.tile([128, 384], F32, tag="mm", bufs=1)
                for c in range(DC):
                    nc.tensor.matmul(
                        pv[:], kgt[c][:, kc * 128:(kc + 1) * 128],
                        wv_sb[c][:, n * 384:(n + 1) * 384],
                        start=(c == 0), stop=(c == DC - 1))
                # copy out with the per-head exp(beta*sim) factor folded in:
                # one broadcast multiply over all 6 heads ([128, 6, 64] view)
                vs = v_sb[kc][:, n * 384:(n + 1) * 384].copy()
                vs.ap = vs.ap[:-1] + [[64, 6], [1, 64]]
                pvr = pv[:].copy()
                pvr.ap = pvr.ap[:-1] + [[64, 6], [1, 64]]
                ebr = ebias[:, kc * H + n * 6:kc * H + n * 6 + 6].copy()
                ebr.ap = ebr.ap + [[0, 64]]
                nc.vector.tensor_mul(vs, pvr, ebr)

        for j in range(NPAIR):
            avden_pair(j, e_w0[j], ots[j])
        qt = prep_window(1)
        outproj_window(0, ots)

        # ------- windows 1..3: steady software pipeline -------
        for tc4 in range(1, NTW):
            ots = [big.tile([128, TW], BF16, tag=f"ot{j}", bufs=2, name="otw")
                   for j in range(NPAIR)]
            for j in range(NPAIR):
                e_j = scores_pair(j, qt)
                avden_pair(j, e_j, ots[j])
            if tc4 + 1 < NTW:
                qt_next = prep_window(tc4 + 1)
            else:
                qt_next = None
            outproj_window(tc4 * (TW // 128), ots)
            qt = qt_next

    nc.compile()
    return nc


def _get_program():
    if "nc" not in _CACHE:
        _CACHE["nc"] = _build_program()
    return _CACHE["nc"]


def _host_prep(inputs):
    import ml_dtypes
    bf16 = ml_dtypes.bfloat16

    f32 = lambda x: np.ascontiguousarray(np.asarray(x, dtype=np.float32))
    Wq, Wk, Wv, Wo = (f32(inputs[k]) for k in ("Wq", "Wk", "Wv", "Wo"))
    bq, bk, bv, bo = (f32(inputs[k]) for k in ("bq", "bk", "bv", "bo"))
    beta = f32(inputs["beta"])

    shared = {
        "wqt": np.ascontiguousarray((Wq.T * SCALE).astype(bf16)),
        "wkt": np.ascontiguousarray(Wk.T.astype(bf16)),
        "wvt": np.ascontiguousarray(Wv.T.astype(bf16)),
        "wot": np.ascontiguousarray(Wo.T.astype(bf16)),
        "bq": np.ascontiguousarray((bq * SCALE).reshape(DC, 128).T),
        "bk": np.ascontiguousarray(bk.reshape(DC, 128).T),
        # bv folded through Wo (sum_k softmax == 1), bo absorbed:
        "bo": np.ascontiguousarray((bo + bv @ Wo.T).reshape(1, D)),
        "beta": np.ascontiguousarray(beta.reshape(1, H)),
    }

    hs = np.asarray(inputs["hidden_states"], dtype=np.float32).astype(bf16)
    kgk = np.asarray(inputs["kg_key"], dtype=np.float32).astype(bf16)
    kgv = np.asarray(inputs["kg_value"], dtype=np.float32).astype(bf16)
    pooled = np.asarray(
        inputs["pooled_hidden_states"], dtype=np.float32).astype(bf16)

    in_maps = []
    for b in range(BS):
        m = dict(shared)
        m["hs"] = np.ascontiguousarray(hs[b])
        m["kgk"] = np.ascontiguousarray(kgk[b])
        m["kgv"] = np.ascontiguousarray(kgv[b])
        m["pooled"] = np.ascontiguousarray(pooled[b].reshape(1, D))
        in_maps.append(m)
    return in_maps




def _install_ntff_hook():
    """Register the axon NTFF profile hook so trace=True yields exec_time_ns.

    Only used from our own test harness (TRACE=True); the default kernel()
    path never calls this.
    """
    try:
        from antenv.axon_hooks import get_axon_ntff_profile_hook  # noqa: F401
        return
    except ImportError:
        pass
    import contextlib
    import ctypes
    import types

    so_path = "/opt/axon/libaxon_pjrt.so"
    try:
        lib = ctypes.CDLL(so_path)
    except OSError:
        return
    if not hasattr(lib, "axon_start_nrt_profile"):
        return
    lib.axon_start_nrt_profile.argtypes = [
        ctypes.POINTER(ctypes.c_int64), ctypes.c_size_t]
    lib.axon_start_nrt_profile.restype = ctypes.c_int64
    lib.axon_stop_nrt_profile.argtypes = [ctypes.c_char_p]
    lib.axon_stop_nrt_profile.restype = ctypes.c_int64

    @contextlib.contextmanager
    def _hook(output_dir, device_ids):
        import jax
        jax.devices()
        if device_ids:
            ids = (ctypes.c_int64 * len(device_ids))(*device_ids)
            rc = lib.axon_start_nrt_profile(ids, len(device_ids))
        else:
            rc = lib.axon_start_nrt_profile(None, 0)
        if rc != 0:
            raise RuntimeError(f"axon_start_nrt_profile rc={rc}")
        try:
            yield
        finally:
            n = lib.axon_stop_nrt_profile(str(output_dir).encode())
            print(f"profile: {n} file(s) written to {output_dir}",
                  file=sys.stderr)

    mod = types.ModuleType("antenv.axon_hooks")
    mod.get_axon_ntff_profile_hook = lambda: _hook
    mod.set_axon_ntff_profile_hook = lambda h: None
    sys.modules["antenv.axon_hooks"] = mod


def kernel(**inputs):
    global LAST_EXEC_NS
    _ensure_path()
    from concourse import bass_utils

    if TRACE:
        _install_ntff_hook()
    nc = _get_program()
    in_maps = _host_prep(inputs)
    res = bass_utils.run_bass_kernel_spmd(
        nc, in_maps, core_ids=list(range(BS)), trace=TRACE)
    LAST_EXEC_NS = res.exec_time_ns
    out = np.stack([res.results[b]["out"] for b in range(BS)], axis=0)
    return out.astype(np.float32)



# revision 42
# speedup vs baseline: 1.1756x; 1.1756x over previous
"""Trainium2 Bass kernel for nn_KnowledgeAttention.

Math (per batch example b):
    sim[k]  = cos_sim(pooled[b], kg_key[b,k])                      # [K]
    q       = (hs @ Wq.T + bq) * HD**-0.5     -> heads [T,H,HD]
    k       = kg_value @ Wk.T + bk            -> heads [K,H,HD]
    v       = kg_value @ Wv.T + bv            -> heads [K,H,HD]
    S[h,t,k]= q_h[t]·k_h[k] + beta[h]*sim[k]
    P       = softmax_k(S);  O[t,h] = sum_k P v
    out     = O @ Wo.T + bo

Sharding: pure data-parallel over batch — 8 examples on 8 cores, weights
replicated, no collectives.

Per-core layout strategy (all matmul contractions run on the partition dim):
    hs.T, kg_value.T via PE transpose; q.T/k.T/v from projections;
    scores computed transposed S.T[k,t] so the cosine-sim bias is a
    per-partition scalar folded into the ACT exp bias; attention output
    O.T[d,t] feeds the final projection lhsT directly; softmax denominators
    via ones-matmuls; normalization uses a gpsimd partition-broadcast of the
    reciprocal row. Matmuls in bf16 with fp32 PSUM accumulation.
"""

import sys

import numpy as np

# ---------------------------------------------------------------- constants
BS = 8
T = 2048
D = 768
H = 12
HD = 64
K = 512
SCALE = HD ** -0.5
EPS = 1e-8
DC = D // 128   # 6 contraction/partition chunks of 128 over D
KC = K // 128   # 4 chunks over K
TW = 512        # t window for moving operand
NTW = T // TW   # 4
NPAIR = H // 2  # 6 head pairs

TRACE = False
LAST_EXEC_NS = None

_CACHE = {}


def _ensure_path():
    try:
        import concourse  # noqa: F401
    except ImportError:
        for p in ("/opt/trn_rl_repo", "/root/.axon_site/_ro/trn_rl_repo"):
            if p not in sys.path:
                sys.path.insert(0, p)


def _build_program():
    _ensure_path()
    import concourse.bass as bass
    import concourse.mybir as mybir
    import concourse.tile as tile
    from concourse import bacc
    from concourse.masks import make_identity
    from contextlib import ExitStack

    F32 = mybir.dt.float32
    BF16 = mybir.dt.bfloat16
    Alu = mybir.AluOpType
    Act = mybir.ActivationFunctionType

    nc = bacc.Bacc("TRN2", target_bir_lowering=False, debug=False, num_devices=BS)

    hs_d = nc.dram_tensor("hs", [T, D], BF16, kind="ExternalInput").ap()
    kgk_d = nc.dram_tensor("kgk", [K, D], BF16, kind="ExternalInput").ap()
    kgv_d = nc.dram_tensor("kgv", [K, D], BF16, kind="ExternalInput").ap()
    pl_d = nc.dram_tensor("pooled", [1, D], BF16, kind="ExternalInput").ap()
    wqt_d = nc.dram_tensor("wqt", [D, D], BF16, kind="ExternalInput").ap()
    wkt_d = nc.dram_tensor("wkt", [D, D], BF16, kind="ExternalInput").ap()
    wvt_d = nc.dram_tensor("wvt", [D, D], BF16, kind="ExternalInput").ap()
    wot_d = nc.dram_tensor("wot", [D, D], BF16, kind="ExternalInput").ap()
    bq_d = nc.dram_tensor("bq", [128, DC], F32, kind="ExternalInput").ap()
    bk_d = nc.dram_tensor("bk", [128, DC], F32, kind="ExternalInput").ap()
    bo_d = nc.dram_tensor("bo", [1, D], F32, kind="ExternalInput").ap()
    beta_d = nc.dram_tensor("beta", [1, H], F32, kind="ExternalInput").ap()
    out_d = nc.dram_tensor("out", [T, D], F32, kind="ExternalOutput").ap()

    with tile.TileContext(nc) as tc, ExitStack() as ctx:
        const = ctx.enter_context(tc.tile_pool(name="const", bufs=1))
        inp = ctx.enter_context(tc.tile_pool(name="inp", bufs=6))
        wpool = ctx.enter_context(tc.tile_pool(name="w", bufs=12))
        big = ctx.enter_context(tc.tile_pool(name="big", bufs=12))
        hstw_p = ctx.enter_context(tc.tile_pool(name="hstw", bufs=12))
        kt_p = ctx.enter_context(tc.tile_pool(name="ktp", bufs=6))
        v_p = ctx.enter_context(tc.tile_pool(name="vp", bufs=4))
        kgt_p = ctx.enter_context(tc.tile_pool(name="kgtp", bufs=6))
        e_p = ctx.enter_context(tc.tile_pool(name="ep", bufs=12))
        r_p = ctx.enter_context(tc.tile_pool(name="rp", bufs=4))
        rb_p = ctx.enter_context(tc.tile_pool(name="rbp", bufs=2))
        fin_p = ctx.enter_context(tc.tile_pool(name="finp", bufs=2))
        sm_p = ctx.enter_context(tc.tile_pool(name="smp", bufs=4))
        ps = ctx.enter_context(tc.tile_pool(name="ps", bufs=2, space="PSUM"))

        # ---------------- phase 0: constants + cosine-sim bias ----------------
        ident = const.tile([128, 128], BF16, tag="ident")
        make_identity(nc, ident[:])
        ones_bf = const.tile([128, 64], BF16, tag="ones_bf")
        nc.vector.memset(ones_bf[:], 1.0)
        # kg_value loads first: transposes are the critical path into phase 1
        kv_tiles = []
        for c in range(KC):
            kv = inp.tile([128, D], BF16, tag="inp", name="kv")
            nc.sync.dma_start(kv[:], kgv_d[c * 128:(c + 1) * 128, :])
            kv_tiles.append(kv)
        wk_sb = []
        for c in range(DC):
            wk = wpool.tile([128, D], BF16, tag="w")
            nc.sync.dma_start(wk[:], wkt_d[c * 128:(c + 1) * 128, :])
            wk_sb.append(wk)
        kk_tiles = []
        for c in range(KC):
            kk = inp.tile([128, D], BF16, tag="kgk", bufs=4, name="kk")
            nc.sync.dma_start(kk[:], kgk_d[c * 128:(c + 1) * 128, :])
            kk_tiles.append(kk)

        pl = const.tile([1, D], BF16, tag="pl")
        nc.sync.dma_start(pl[:], pl_d)
        bt = const.tile([1, H], F32, tag="bt")
        nc.sync.dma_start(bt[:], beta_d)
        bo_row = const.tile([1, D], F32, tag="bo_row")
        nc.sync.dma_start(bo_row[:], bo_d)
        bq_sb = const.tile([128, DC], F32, tag="bq_sb")
        nc.sync.dma_start(bq_sb[:], bq_d)
        bk_sb = const.tile([128, DC], F32, tag="bk_sb")
        nc.sync.dma_start(bk_sb[:], bk_d)

        bo_bc = const.tile([128, D], F32, tag="bo_bc")
        nc.gpsimd.partition_broadcast(bo_bc[:], bo_row[:], channels=128)
        beta_bc = const.tile([128, H], F32, tag="beta_bc")
        nc.gpsimd.partition_broadcast(beta_bc[:], bt[:], channels=128)
        pl_bc = const.tile([128, D], BF16, tag="pl_bc")
        nc.gpsimd.partition_broadcast(pl_bc[:], pl[:], channels=128)

        # pooled 1/||.|| as a per-partition vector (computed on the broadcast)
        pl_sq = inp.tile([128, D], BF16, tag="inp", name="pl_sq")
        pnorm = sm_p.tile([128, 1], F32, tag="pnorm")
        nc.scalar.activation(pl_sq[:], pl_bc[:], Act.Square, accum_out=pnorm[:])
        nc.scalar.activation(pnorm[:], pnorm[:], Act.Sqrt)
        nc.vector.tensor_scalar_max(pnorm[:], pnorm[:], EPS)
        rp_vec = const.tile([128, 1], F32, tag="rp_vec")
        nc.vector.reciprocal(rp_vec[:], pnorm[:])

        # ---------------- phase 1a: kg_value.T, k.T ----------------
        # remaining weight DMAs in consumption order: wq (prep0), wv, wo
        wq_sb = []
        for c in range(DC):
            wq = wpool.tile([128, D], BF16, tag="w")
            nc.sync.dma_start(wq[:], wqt_d[c * 128:(c + 1) * 128, :])
            wq_sb.append(wq)
        hv0_tiles = []
        for tsub in range(TW // 128):
            hv = inp.tile([128, D], BF16, tag="inp", name="hv")
            nc.sync.dma_start(hv[:], hs_d[tsub * 128:(tsub + 1) * 128, :])
            hv0_tiles.append(hv)

        kgt = [kgt_p.tile([128, K], BF16, tag="kgt", name="kgt") for _ in range(DC)]
        for dchunk in range(DC):
            pt = ps.tile([128, K], BF16, tag="mm", bufs=2, name="ptr")
            for c in range(KC):
                nc.tensor.transpose(
                    pt[:, c * 128:(c + 1) * 128],
                    kv_tiles[c][:, dchunk * 128:(dchunk + 1) * 128], ident[:])
            nc.vector.tensor_copy(kgt[dchunk][:], pt[:])

        kt = [kt_p.tile([128, K], BF16, tag="kt", name="kt") for _ in range(DC)]
        for m in range(DC):
            pk = ps.tile([128, K], F32, tag="mm", bufs=2)
            for c in range(DC):
                nc.tensor.matmul(
                    pk[:], wk_sb[c][:, m * 128:(m + 1) * 128], kgt[c][:],
                    start=(c == 0), stop=(c == DC - 1))
            nc.vector.tensor_scalar_add(kt[m][:], pk[:], bk_sb[:, m:m + 1])

        def prep_window(tc4, hv_tiles=None):
            hstw = [hstw_p.tile([128, TW], BF16, tag="hstw", name="hstw")
                    for _ in range(DC)]
            if hv_tiles is None:
                hv_tiles = []
                for tsub in range(TW // 128):
                    hv = inp.tile([128, D], BF16, tag="inp", name="hv")
                    t0 = tc4 * TW + tsub * 128
                    nc.sync.dma_start(hv[:], hs_d[t0:t0 + 128, :])
                    hv_tiles.append(hv)
            for c in range(DC):
                pt = ps.tile([128, TW], BF16, tag="mm", bufs=2, name="ptr")
                for tsub in range(TW // 128):
                    nc.tensor.transpose(
                        pt[:, tsub * 128:(tsub + 1) * 128],
                        hv_tiles[tsub][:, c * 128:(c + 1) * 128], ident[:])
                nc.vector.tensor_copy(hstw[c][:], pt[:])
            qts = [big.tile([128, TW], BF16, tag=f"qt{m}", bufs=2, name="qtw")
                   for m in range(DC)]
            for m in range(DC):
                pq = ps.tile([128, TW], F32, tag="mm", bufs=2)
                for c in range(DC):
                    nc.tensor.matmul(
                        pq[:], wq_sb[c][:, m * 128:(m + 1) * 128], hstw[c][:],
                        start=(c == 0), stop=(c == DC - 1))
                nc.vector.tensor_scalar_add(qts[m][:], pq[:], bq_sb[:, m:m + 1])
            return qts

        qt = prep_window(0, hv0_tiles)

        # ------- cosine-sim bias + ebias (ACT/DVE, overlaps PE prep) -------
        bias_all = const.tile([128, KC * H], F32, tag="bias_all")
        for c in range(KC):
            kk = kk_tiles[c]
            sq = inp.tile([128, D], BF16, tag="inp")
            nrm = sm_p.tile([128, 1], F32, tag="nrm")
            nc.scalar.activation(sq[:], kk[:], Act.Square, accum_out=nrm[:])
            nc.scalar.activation(nrm[:], nrm[:], Act.Sqrt)
            nc.vector.tensor_scalar_max(nrm[:], nrm[:], EPS)
            rn = sm_p.tile([128, 1], F32, tag="rn")
            nc.vector.reciprocal(rn[:], nrm[:])
            sq2 = inp.tile([128, D], BF16, tag="inp")
            dot = sm_p.tile([128, 1], F32, tag="dot")
            nc.vector.scalar_tensor_tensor(
                out=sq2[:], in0=kk[:], scalar=1.0, in1=pl_bc[:],
                op0=Alu.mult, op1=Alu.mult, accum_out=dot[:])
            nc.vector.tensor_mul(dot[:], dot[:], rn[:])
            nc.vector.tensor_mul(dot[:], dot[:], rp_vec[:])
            nc.vector.tensor_scalar_mul(
                bias_all[:, c * H:(c + 1) * H], beta_bc[:], dot[:])

        # ebias[k_part, kc*H + h] = exp(beta[h]*sim[k]) — folded into v and
        # the denominator weights so the softmax exp needs no bias (enables
        # even/odd-merged [128,1024] exps)
        ebias = const.tile([128, KC * H], F32, tag="ebias")
        nc.scalar.activation(ebias[:], bias_all[:], Act.Exp)
        ebias_bf = const.tile([128, KC * H], BF16, tag="ebias_bf")
        nc.vector.tensor_copy(ebias_bf[:], ebias[:])

        def eb_col64(c, h):
            # [128, 64] broadcast view of ebias_bf column kc*H+h (stride-0)
            col = ebias_bf[:, c * H + h:c * H + h + 1].copy()
            col.ap = col.ap[:-1] + [[0, 64]]
            return col

        # scores: even head on PE row-tile (0,0) -> cols 0:512 of a 2-bank
        # tile, odd head on (64,0) -> cols 512:1024; one bias-free exp
        # covers both banks.
        def scores_pair(j, qtw, ts0=0, w=TW):
            # odd head always lands at the TW (bank-aligned) offset; for
            # w < TW one exp covers [0, TW+w) and cols [w, TW) are unused.
            e_j = []
            for kc in range(KC):
                psc = ps.tile([128, TW + w], F32, tag="s", bufs=2)
                nc.tensor.matmul(
                    psc[:, 0:w], kt[j][0:64, kc * 128:(kc + 1) * 128],
                    qtw[j][0:64, ts0:ts0 + w], start=True, stop=True)
                nc.tensor.matmul(
                    psc[:, TW:TW + w],
                    kt[j][64:128, kc * 128:(kc + 1) * 128],
                    qtw[j][64:128, ts0:ts0 + w], start=True, stop=True)
                ebig = e_p.tile([128, 2 * w], BF16, tag="e", bufs=16)
                src = psc[:].copy()
                src.ap = src.ap[:-1] + [[TW, 2], [1, w]]
                dst = ebig[:].copy()
                dst.ap = dst.ap[:-1] + [[w, 2], [1, w]]
                nc.scalar.activation(dst, src, Act.Exp)
                e_j.append(ebig)
            return e_j

        # AV + denominators: even head -> col-tile (0,0) rows 0:64, odd head
        # -> col-tile (0,64) rows 64:128 of the same bank; interleaved issue
        # so both column tiles stream concurrently. pd rows 0:64 = sum_k
        # e_even (64x replicated), rows 64:128 = odd.
        def avden_pair(j, e_j, ot_j, w=TW):
            po = ps.tile([128, w], F32, tag="o", bufs=1, name="po")
            pd = ps.tile([128, w], F32, tag="d", bufs=1, name="pd")
            for kc in range(KC):
                st = (kc == 0)
                sp = (kc == KC - 1)
                nc.tensor.matmul(
                    po[0:64, :],
                    v_sb[kc][:, (2 * j) * HD:(2 * j + 1) * HD],
                    e_j[kc][:, 0:w], start=st, stop=sp)
                nc.tensor.matmul(
                    po[64:128, :],
                    v_sb[kc][:, (2 * j + 1) * HD:(2 * j + 2) * HD],
                    e_j[kc][:, w:2 * w], start=st, stop=sp)
                nc.tensor.matmul(
                    pd[0:64, :], eb_col64(kc, 2 * j),
                    e_j[kc][:, 0:w], start=st, stop=sp)
                nc.tensor.matmul(
                    pd[64:128, :], eb_col64(kc, 2 * j + 1),
                    e_j[kc][:, w:2 * w], start=st, stop=sp)
            rall = r_p.tile([128, w], F32, tag="rall", name="rall")
            # high priority: these free the po/pd banks — keep them ahead
            # of filler DVE work so the next pair's AV/den can start
            with tc.high_priority():
                nc.vector.reciprocal_approx_fast(rall[:], pd[:])
                nc.vector.tensor_mul(ot_j[:], po[:], rall[:])

        def outproj_window(tc16_0, ots, w=TW):
            for tsub in range(w // 128):
                tc16 = tc16_0 + tsub
                fin = fin_p.tile([128, D], F32, tag="fin")
                for n in range(2):
                    pf = ps.tile([128, 384], F32, tag="mm", bufs=2)
                    for c in range(DC):
                        nc.tensor.matmul(
                            pf[:], ots[c][:, tsub * 128:(tsub + 1) * 128],
                            wo_sb[c][:, n * 384:(n + 1) * 384],
                            start=(c == 0), stop=(c == DC - 1))
                    nc.vector.tensor_add(
                        fin[:, n * 384:(n + 1) * 384], pf[:],
                        bo_bc[:, n * 384:(n + 1) * 384])
                nc.sync.dma_start(out_d[tc16 * 128:(tc16 + 1) * 128, :], fin[:])

        # ------- window 0: all scores issued first; the v projection below
        # is the PE filler for the exp-paced stretch; AV/den then consume.
        ots = [big.tile([128, TW], BF16, tag=f"ot{j}", bufs=2, name="otw")
               for j in range(NPAIR)]
        e_w0 = [scores_pair(j, qt) for j in range(NPAIR)]

        # ---------------- v projection (+ ebias fold) ----------------
        wv_sb = []
        for c in range(DC):
            wv = wpool.tile([128, D], BF16, tag="w")
            nc.sync.dma_start(wv[:], wvt_d[c * 128:(c + 1) * 128, :])
            wv_sb.append(wv)
        wo_sb = []
        for c in range(DC):
            wo = wpool.tile([128, D], BF16, tag="w")
            nc.sync.dma_start(wo[:], wot_d[c * 128:(c + 1) * 128, :])
            wo_sb.append(wo)

        v_sb = [v_p.tile([128, D], BF16, tag="v", name="vsb")
                for _ in range(KC)]
        for kc in range(KC):
            for n in range(2):
                pv = ps.tile([128, 384], F32, tag="mm", bufs=2)
                for c in range(DC):
                    nc.tensor.matmul(
                        pv[:], kgt[c][:, kc * 128:(kc + 1) * 128],
                        wv_sb[c][:, n * 384:(n + 1) * 384],
                        start=(c == 0), stop=(c == DC - 1))
                # copy out with the per-head exp(beta*sim) factor folded in:
                # one broadcast multiply over all 6 heads ([128, 6, 64] view)
                vs = v_sb[kc][:, n * 384:(n + 1) * 384].copy()
                vs.ap = vs.ap[:-1] + [[64, 6], [1, 64]]
                pvr = pv[:].copy()
                pvr.ap = pvr.ap[:-1] + [[64, 6], [1, 64]]
                ebr = ebias[:, kc * H + n * 6:kc * H + n * 6 + 6].copy()
                ebr.ap = ebr.ap + [[0, 64]]
                nc.vector.tensor_mul(vs, pvr, ebr)

        for j in range(NPAIR):
            avden_pair(j, e_w0[j], ots[j])
        qt = prep_window(1)
        outproj_window(0, ots)

        # ------- windows 1..3: steady software pipeline -------
        for tc4 in range(1, NTW):
            ots = [big.tile([128, TW], BF16, tag=f"ot{j}", bufs=2, name="otw")
                   for j in range(NPAIR)]
            for j in range(NPAIR):
                e_j = scores_pair(j, qt)
                avden_pair(j, e_j, ots[j])
            if tc4 + 1 < NTW:
                qt_next = prep_window(tc4 + 1)
            else:
                qt_next = None
            outproj_window(tc4 * (TW // 128), ots)
            qt = qt_next

    nc.compile()
    return nc


def _get_program():
    if "nc" not in _CACHE:
        _CACHE["nc"] = _build_program()
    return _CACHE["nc"]


def _host_prep(inputs):
    import ml_dtypes
    bf16 = ml_dtypes.bfloat16

    f32 = lambda x: np.ascontiguousarray(np.asarray(x, dtype=np.float32))
    Wq, Wk, Wv, Wo = (f32(inputs[k]) for k in ("Wq", "Wk", "Wv", "Wo"))
    bq, bk, bv, bo = (f32(inputs[k]) for k in ("bq", "bk", "bv", "bo"))
    beta = f32(inputs["beta"])

    shared = {
        "wqt": np.ascontiguousarray((Wq.T * SCALE).astype(bf16)),
        "wkt": np.ascontiguousarray(Wk.T.astype(bf16)),
        "wvt": np.ascontiguousarray(Wv.T.astype(bf16)),
        "wot": np.ascontiguousarray(Wo.T.astype(bf16)),
        "bq": np.ascontiguousarray((bq * SCALE).reshape(DC, 128).T),
        "bk": np.ascontiguousarray(bk.reshape(DC, 128).T),
        # bv folded through Wo (sum_k softmax == 1), bo absorbed:
        "bo": np.ascontiguousarray((bo + bv @ Wo.T).reshape(1, D)),
        "beta": np.ascontiguousarray(beta.reshape(1, H)),
    }

    hs = np.asarray(inputs["hidden_states"], dtype=np.float32).astype(bf16)
    kgk = np.asarray(inputs["kg_key"], dtype=np.float32).astype(bf16)
    kgv = np.asarray(inputs["kg_value"], dtype=np.float32).astype(bf16)
    pooled = np.asarray(
        inputs["pooled_hidden_states"], dtype=np.float32).astype(bf16)

    in_maps = []
    for b in range(BS):
        m = dict(shared)
        m["hs"] = np.ascontiguousarray(hs[b])
        m["kgk"] = np.ascontiguousarray(kgk[b])
        m["kgv"] = np.ascontiguousarray(kgv[b])
        m["pooled"] = np.ascontiguousarray(pooled[b].reshape(1, D))
        in_maps.append(m)
    return in_maps




def _install_ntff_hook():
    """Register the axon NTFF profile hook so trace=True yields exec_time_ns.

    Only used from our own test harness (TRACE=True); the default kernel()
    path never calls this.
    """
    try:
        from antenv.axon_hooks import get_axon_ntff_profile_hook  # noqa: F401
        return
    except ImportError:
        pass
    import contextlib
    import ctypes
    import types

    so_path = "/opt/axon/libaxon_pjrt.so"
    try:
        lib = ctypes.CDLL(so_path)
    except OSError:
        return
    if not hasattr(lib, "axon_start_nrt_profile"):
        return
    lib.axon_start_nrt_profile.argtypes = [
        ctypes.POINTER(ctypes.c_int64), ctypes.c_size_t]
    lib.axon_start_nrt_profile.restype = ctypes.c_int64
    lib.axon_stop_nrt_profile.argtypes = [ctypes.c_char_p]
    lib.axon_stop_nrt_profile.restype = ctypes.c_int64

    @contextlib.contextmanager
    def _hook(output_dir, device_ids):
        import jax
        jax.devices()
        if device_ids:
            ids = (ctypes.c_int64 * len(device_ids))(*device_ids)
            rc = lib.axon_start_nrt_profile(ids, len(device_ids))
        else:
            rc = lib.axon_start_nrt_profile(None, 0)
        if rc != 0:
            raise RuntimeError(f"axon_start_nrt_profile rc={rc}")
        try:
            yield
        finally:
            n = lib.axon_stop_nrt_profile(str(output_dir).encode())
            print(f"profile: {n} file(s) written to {output_dir}",
                  file=sys.stderr)

    mod = types.ModuleType("antenv.axon_hooks")
    mod.get_axon_ntff_profile_hook = lambda: _hook
    mod.set_axon_ntff_profile_hook = lambda h: None
    sys.modules["antenv.axon_hooks"] = mod


def kernel(**inputs):
    global LAST_EXEC_NS
    _ensure_path()
    from concourse import bass_utils

    if TRACE:
        _install_ntff_hook()
    nc = _get_program()
    in_maps = _host_prep(inputs)
    res = bass_utils.run_bass_kernel_spmd(
        nc, in_maps, core_ids=list(range(BS)), trace=TRACE)
    LAST_EXEC_NS = res.exec_time_ns
    out = np.stack([res.results[b]["out"] for b in range(BS)], axis=0)
    return out.astype(np.float32)



# revision 43
# speedup vs baseline: 1.1966x; 1.0178x over previous
"""Trainium2 Bass kernel for nn_KnowledgeAttention.

Math (per batch example b):
    sim[k]  = cos_sim(pooled[b], kg_key[b,k])                      # [K]
    q       = (hs @ Wq.T + bq) * HD**-0.5     -> heads [T,H,HD]
    k       = kg_value @ Wk.T + bk            -> heads [K,H,HD]
    v       = kg_value @ Wv.T + bv            -> heads [K,H,HD]
    S[h,t,k]= q_h[t]·k_h[k] + beta[h]*sim[k]
    P       = softmax_k(S);  O[t,h] = sum_k P v
    out     = O @ Wo.T + bo

Sharding: pure data-parallel over batch — 8 examples on 8 cores, weights
replicated, no collectives.

Per-core strategy (contractions on the partition dim, all matmuls bf16 with
fp32 PSUM accumulation):
  - hs/kg inputs cast to bf16 on the host (halves input DMA); hs.T and
    kg_value.T via bf16 PE transposes.
  - The softmax bias exp(beta_h*sim[k]) is folded multiplicatively into v
    (during the V-projection PSUM copy) and into the denominator weights
    (stride-0 broadcast APs over ebias columns), so the exps are bias-free.
  - Scores per head pair: even head on PE row-tile (0,0), odd on (64,0) —
    the two half-array matmuls stream concurrently into one 2-bank PSUM
    tile covered by a single [128,1024] exp.
  - AV and 64x-replicated denominators: even head on col-tile (0,0) rows
    0:64, odd on (0,64) rows 64:128 — column pairs stream concurrently;
    softmax normalization is one reciprocal + one multiply per pair.
  - Software-pipelined windows: prep(w+1) (hs DMA/transpose/Q-proj) and
    out-proj(w) are issued after attention(w) so the scheduler uses them
    as PE filler during the exp-paced stretches.
  - PSUM rings: scores 2x[128,1024], AV 1, denom 1, everything else 2.
"""

import sys

import numpy as np

# ---------------------------------------------------------------- constants
BS = 8
T = 2048
D = 768
H = 12
HD = 64
K = 512
SCALE = HD ** -0.5
EPS = 1e-8
DC = D // 128   # 6 contraction/partition chunks of 128 over D
KC = K // 128   # 4 chunks over K
TW = 512        # t window for moving operand
NTW = T // TW   # 4
NPAIR = H // 2  # 6 head pairs

TRACE = False
LAST_EXEC_NS = None

_CACHE = {}


def _ensure_path():
    try:
        import concourse  # noqa: F401
    except ImportError:
        for p in ("/opt/trn_rl_repo", "/root/.axon_site/_ro/trn_rl_repo"):
            if p not in sys.path:
                sys.path.insert(0, p)


def _build_program():
    _ensure_path()
    import concourse.bass as bass
    import concourse.mybir as mybir
    import concourse.tile as tile
    from concourse import bacc
    from concourse.masks import make_identity
    from contextlib import ExitStack

    F32 = mybir.dt.float32
    BF16 = mybir.dt.bfloat16
    Alu = mybir.AluOpType
    Act = mybir.ActivationFunctionType

    nc = bacc.Bacc("TRN2", target_bir_lowering=False, debug=False, num_devices=BS)

    hs_d = nc.dram_tensor("hs", [T, D], BF16, kind="ExternalInput").ap()
    kgk_d = nc.dram_tensor("kgk", [K, D], BF16, kind="ExternalInput").ap()
    kgv_d = nc.dram_tensor("kgv", [K, D], BF16, kind="ExternalInput").ap()
    pl_d = nc.dram_tensor("pooled", [1, D], BF16, kind="ExternalInput").ap()
    wqt_d = nc.dram_tensor("wqt", [D, D], BF16, kind="ExternalInput").ap()
    wkt_d = nc.dram_tensor("wkt", [D, D], BF16, kind="ExternalInput").ap()
    wvt_d = nc.dram_tensor("wvt", [D, D], BF16, kind="ExternalInput").ap()
    wot_d = nc.dram_tensor("wot", [D, D], BF16, kind="ExternalInput").ap()
    bq_d = nc.dram_tensor("bq", [128, DC], F32, kind="ExternalInput").ap()
    bk_d = nc.dram_tensor("bk", [128, DC], F32, kind="ExternalInput").ap()
    bo_d = nc.dram_tensor("bo", [1, D], F32, kind="ExternalInput").ap()
    beta_d = nc.dram_tensor("beta", [1, H], F32, kind="ExternalInput").ap()
    out_d = nc.dram_tensor("out", [T, D], F32, kind="ExternalOutput").ap()

    with tile.TileContext(nc) as tc, ExitStack() as ctx:
        const = ctx.enter_context(tc.tile_pool(name="const", bufs=1))
        inp = ctx.enter_context(tc.tile_pool(name="inp", bufs=6))
        wpool = ctx.enter_context(tc.tile_pool(name="w", bufs=12))
        big = ctx.enter_context(tc.tile_pool(name="big", bufs=12))
        hstw_p = ctx.enter_context(tc.tile_pool(name="hstw", bufs=12))
        kt_p = ctx.enter_context(tc.tile_pool(name="ktp", bufs=6))
        v_p = ctx.enter_context(tc.tile_pool(name="vp", bufs=4))
        kgt_p = ctx.enter_context(tc.tile_pool(name="kgtp", bufs=6))
        e_p = ctx.enter_context(tc.tile_pool(name="ep", bufs=12))
        r_p = ctx.enter_context(tc.tile_pool(name="rp", bufs=4))
        rb_p = ctx.enter_context(tc.tile_pool(name="rbp", bufs=2))
        fin_p = ctx.enter_context(tc.tile_pool(name="finp", bufs=2))
        sm_p = ctx.enter_context(tc.tile_pool(name="smp", bufs=4))
        ps = ctx.enter_context(tc.tile_pool(name="ps", bufs=2, space="PSUM"))

        # ---------------- phase 0: constants + cosine-sim bias ----------------
        ident = const.tile([128, 128], BF16, tag="ident")
        make_identity(nc, ident[:])
        ones_bf = const.tile([128, 64], BF16, tag="ones_bf")
        nc.vector.memset(ones_bf[:], 1.0)
        # kg_value loads first: transposes are the critical path into phase 1
        kv_tiles = []
        for c in range(KC):
            kv = inp.tile([128, D], BF16, tag="inp", name="kv")
            nc.sync.dma_start(kv[:], kgv_d[c * 128:(c + 1) * 128, :])
            kv_tiles.append(kv)
        wk_sb = []
        for c in range(DC):
            wk = wpool.tile([128, D], BF16, tag="w")
            nc.sync.dma_start(wk[:], wkt_d[c * 128:(c + 1) * 128, :])
            wk_sb.append(wk)
        kk_tiles = []
        for c in range(KC):
            kk = inp.tile([128, D], BF16, tag="kgk", bufs=4, name="kk")
            nc.sync.dma_start(kk[:], kgk_d[c * 128:(c + 1) * 128, :])
            kk_tiles.append(kk)

        pl = const.tile([1, D], BF16, tag="pl")
        nc.sync.dma_start(pl[:], pl_d)
        bt = const.tile([1, H], F32, tag="bt")
        nc.sync.dma_start(bt[:], beta_d)
        bo_row = const.tile([1, D], F32, tag="bo_row")
        nc.sync.dma_start(bo_row[:], bo_d)
        bq_sb = const.tile([128, DC], F32, tag="bq_sb")
        nc.sync.dma_start(bq_sb[:], bq_d)
        bk_sb = const.tile([128, DC], F32, tag="bk_sb")
        nc.sync.dma_start(bk_sb[:], bk_d)

        bo_bc = const.tile([128, D], F32, tag="bo_bc")
        nc.gpsimd.partition_broadcast(bo_bc[:], bo_row[:], channels=128)
        beta_bc = const.tile([128, H], F32, tag="beta_bc")
        nc.gpsimd.partition_broadcast(beta_bc[:], bt[:], channels=128)
        pl_bc = const.tile([128, D], BF16, tag="pl_bc")
        nc.gpsimd.partition_broadcast(pl_bc[:], pl[:], channels=128)

        # pooled 1/||.|| as a per-partition vector (computed on the broadcast)
        pl_sq = inp.tile([128, D], BF16, tag="inp", name="pl_sq")
        pnorm = sm_p.tile([128, 1], F32, tag="pnorm")
        nc.scalar.activation(pl_sq[:], pl_bc[:], Act.Square, accum_out=pnorm[:])
        nc.scalar.activation(pnorm[:], pnorm[:], Act.Sqrt)
        nc.vector.tensor_scalar_max(pnorm[:], pnorm[:], EPS)
        rp_vec = const.tile([128, 1], F32, tag="rp_vec")
        nc.vector.reciprocal(rp_vec[:], pnorm[:])

        # ---------------- phase 1a: kg_value.T, k.T ----------------
        # remaining weight DMAs in consumption order: wq (prep0), wv, wo
        wq_sb = []
        for c in range(DC):
            wq = wpool.tile([128, D], BF16, tag="w")
            nc.sync.dma_start(wq[:], wqt_d[c * 128:(c + 1) * 128, :])
            wq_sb.append(wq)
        hv0_tiles = []
        for tsub in range(TW // 128):
            hv = inp.tile([128, D], BF16, tag="inp", name="hv")
            nc.sync.dma_start(hv[:], hs_d[tsub * 128:(tsub + 1) * 128, :])
            hv0_tiles.append(hv)

        kgt = [kgt_p.tile([128, K], BF16, tag="kgt", name="kgt") for _ in range(DC)]
        for dchunk in range(DC):
            pt = ps.tile([128, K], BF16, tag="mm", bufs=2, name="ptr")
            for c in range(KC):
                nc.tensor.transpose(
                    pt[:, c * 128:(c + 1) * 128],
                    kv_tiles[c][:, dchunk * 128:(dchunk + 1) * 128], ident[:])
            nc.vector.tensor_copy(kgt[dchunk][:], pt[:])

        kt = [kt_p.tile([128, K], BF16, tag="kt", name="kt") for _ in range(DC)]
        for m in range(DC):
            pk = ps.tile([128, K], F32, tag="mm", bufs=2)
            for c in range(DC):
                nc.tensor.matmul(
                    pk[:], wk_sb[c][:, m * 128:(m + 1) * 128], kgt[c][:],
                    start=(c == 0), stop=(c == DC - 1))
            nc.vector.tensor_scalar_add(kt[m][:], pk[:], bk_sb[:, m:m + 1])

        def prep_window(tc4, hv_tiles=None):
            hstw = [hstw_p.tile([128, TW], BF16, tag="hstw", name="hstw")
                    for _ in range(DC)]
            if hv_tiles is None:
                hv_tiles = []
                for tsub in range(TW // 128):
                    hv = inp.tile([128, D], BF16, tag="inp", name="hv")
                    t0 = tc4 * TW + tsub * 128
                    nc.sync.dma_start(hv[:], hs_d[t0:t0 + 128, :])
                    hv_tiles.append(hv)
            for c in range(DC):
                pt = ps.tile([128, TW], BF16, tag="mm", bufs=2, name="ptr")
                for tsub in range(TW // 128):
                    nc.tensor.transpose(
                        pt[:, tsub * 128:(tsub + 1) * 128],
                        hv_tiles[tsub][:, c * 128:(c + 1) * 128], ident[:])
                nc.vector.tensor_copy(hstw[c][:], pt[:])
            qts = [big.tile([128, TW], BF16, tag=f"qt{m}", bufs=2, name="qtw")
                   for m in range(DC)]
            for m in range(DC):
                pq = ps.tile([128, TW], F32, tag="mm", bufs=2)
                for c in range(DC):
                    nc.tensor.matmul(
                        pq[:], wq_sb[c][:, m * 128:(m + 1) * 128], hstw[c][:],
                        start=(c == 0), stop=(c == DC - 1))
                nc.vector.tensor_scalar_add(qts[m][:], pq[:], bq_sb[:, m:m + 1])
            return qts

        qt = prep_window(0, hv0_tiles)

        # ------- cosine-sim bias + ebias (ACT/DVE, overlaps PE prep) -------
        bias_all = const.tile([128, KC * H], F32, tag="bias_all")
        for c in range(KC):
            kk = kk_tiles[c]
            sq = inp.tile([128, D], BF16, tag="inp")
            nrm = sm_p.tile([128, 1], F32, tag="nrm")
            nc.scalar.activation(sq[:], kk[:], Act.Square, accum_out=nrm[:])
            nc.scalar.activation(nrm[:], nrm[:], Act.Sqrt)
            nc.vector.tensor_scalar_max(nrm[:], nrm[:], EPS)
            rn = sm_p.tile([128, 1], F32, tag="rn")
            nc.vector.reciprocal(rn[:], nrm[:])
            sq2 = inp.tile([128, D], BF16, tag="inp")
            dot = sm_p.tile([128, 1], F32, tag="dot")
            nc.vector.scalar_tensor_tensor(
                out=sq2[:], in0=kk[:], scalar=1.0, in1=pl_bc[:],
                op0=Alu.mult, op1=Alu.mult, accum_out=dot[:])
            nc.vector.tensor_mul(dot[:], dot[:], rn[:])
            nc.vector.tensor_mul(dot[:], dot[:], rp_vec[:])
            nc.vector.tensor_scalar_mul(
                bias_all[:, c * H:(c + 1) * H], beta_bc[:], dot[:])

        # ebias[k_part, kc*H + h] = exp(beta[h]*sim[k]) — folded into v and
        # the denominator weights so the softmax exp needs no bias (enables
        # even/odd-merged [128,1024] exps)
        ebias = const.tile([128, KC * H], F32, tag="ebias")
        nc.scalar.activation(ebias[:], bias_all[:], Act.Exp)
        ebias_bf = const.tile([128, KC * H], BF16, tag="ebias_bf")
        nc.vector.tensor_copy(ebias_bf[:], ebias[:])

        def eb_col64(c, h):
            # [128, 64] broadcast view of ebias_bf column kc*H+h (stride-0)
            col = ebias_bf[:, c * H + h:c * H + h + 1].copy()
            col.ap = col.ap[:-1] + [[0, 64]]
            return col

        # scores: even head on PE row-tile (0,0) -> cols 0:512 of a 2-bank
        # tile, odd head on (64,0) -> cols 512:1024; one bias-free exp
        # covers both banks.
        def scores_pair(j, qtw, ts0=0, w=TW):
            # odd head always lands at the TW (bank-aligned) offset; for
            # w < TW one exp covers [0, TW+w) and cols [w, TW) are unused.
            e_j = []
            for kc in range(KC):
                psc = ps.tile([128, TW + w], F32, tag="s", bufs=2)
                nc.tensor.matmul(
                    psc[:, 0:w], kt[j][0:64, kc * 128:(kc + 1) * 128],
                    qtw[j][0:64, ts0:ts0 + w], start=True, stop=True)
                nc.tensor.matmul(
                    psc[:, TW:TW + w],
                    kt[j][64:128, kc * 128:(kc + 1) * 128],
                    qtw[j][64:128, ts0:ts0 + w], start=True, stop=True)
                ebig = e_p.tile([128, 2 * w], BF16, tag="e", bufs=16)
                src = psc[:].copy()
                src.ap = src.ap[:-1] + [[TW, 2], [1, w]]
                dst = ebig[:].copy()
                dst.ap = dst.ap[:-1] + [[w, 2], [1, w]]
                nc.scalar.activation(dst, src, Act.Exp)
                e_j.append(ebig)
            return e_j

        # AV + denominators: even head -> col-tile (0,0) rows 0:64, odd head
        # -> col-tile (0,64) rows 64:128 of the same bank; interleaved issue
        # so both column tiles stream concurrently. pd rows 0:64 = sum_k
        # e_even (64x replicated), rows 64:128 = odd.
        def avden_pair(j, e_j, ot_j, w=TW):
            po = ps.tile([128, w], F32, tag="o", bufs=1, name="po")
            pd = ps.tile([128, w], F32, tag="d", bufs=1, name="pd")
            for kc in range(KC):
                st = (kc == 0)
                sp = (kc == KC - 1)
                nc.tensor.matmul(
                    po[0:64, :],
                    v_sb[kc][:, (2 * j) * HD:(2 * j + 1) * HD],
                    e_j[kc][:, 0:w], start=st, stop=sp)
                nc.tensor.matmul(
                    po[64:128, :],
                    v_sb[kc][:, (2 * j + 1) * HD:(2 * j + 2) * HD],
                    e_j[kc][:, w:2 * w], start=st, stop=sp)
                nc.tensor.matmul(
                    pd[0:64, :], eb_col64(kc, 2 * j),
                    e_j[kc][:, 0:w], start=st, stop=sp)
                nc.tensor.matmul(
                    pd[64:128, :], eb_col64(kc, 2 * j + 1),
                    e_j[kc][:, w:2 * w], start=st, stop=sp)
            rall = r_p.tile([128, w], F32, tag="rall", name="rall")
            # high priority: these free the po/pd banks — keep them ahead
            # of filler DVE work so the next pair's AV/den can start
            with tc.high_priority():
                nc.vector.reciprocal_approx_fast(rall[:], pd[:])
                nc.vector.tensor_mul(ot_j[:], po[:], rall[:])

        def outproj_window(tc16_0, ots, w=TW):
            for tsub in range(w // 128):
                tc16 = tc16_0 + tsub
                fin = fin_p.tile([128, D], F32, tag="fin")
                for n in range(2):
                    pf = ps.tile([128, 384], F32, tag="mm", bufs=2)
                    for c in range(DC):
                        nc.tensor.matmul(
                            pf[:], ots[c][:, tsub * 128:(tsub + 1) * 128],
                            wo_sb[c][:, n * 384:(n + 1) * 384],
                            start=(c == 0), stop=(c == DC - 1))
                    nc.vector.tensor_add(
                        fin[:, n * 384:(n + 1) * 384], pf[:],
                        bo_bc[:, n * 384:(n + 1) * 384])
                nc.sync.dma_start(out_d[tc16 * 128:(tc16 + 1) * 128, :], fin[:])

        # ------- window 0: all scores issued first; the v projection below
        # is the PE filler for the exp-paced stretch; AV/den then consume.
        ots = [big.tile([128, TW], BF16, tag=f"ot{j}", bufs=2, name="otw")
               for j in range(NPAIR)]
        e_w0 = [scores_pair(j, qt) for j in range(NPAIR)]

        # ---------------- v projection (+ ebias fold) ----------------
        wv_sb = []
        for c in range(DC):
            wv = wpool.tile([128, D], BF16, tag="w")
            nc.sync.dma_start(wv[:], wvt_d[c * 128:(c + 1) * 128, :])
            wv_sb.append(wv)
        wo_sb = []
        for c in range(DC):
            wo = wpool.tile([128, D], BF16, tag="w")
            nc.sync.dma_start(wo[:], wot_d[c * 128:(c + 1) * 128, :])
            wo_sb.append(wo)

        v_sb = [v_p.tile([128, D], BF16, tag="v", name="vsb")
                for _ in range(KC)]
        for kc in range(KC):
            for n in range(2):
                pv = ps.tile([128, 384], F32, tag="mm", bufs=2)
                for c in range(DC):
                    nc.tensor.matmul(
                        pv[:], kgt[c][:, kc * 128:(kc + 1) * 128],
                        wv_sb[c][:, n * 384:(n + 1) * 384],
                        start=(c == 0), stop=(c == DC - 1))
                # copy out with the per-head exp(beta*sim) factor folded in:
                # one broadcast multiply over all 6 heads ([128, 6, 64] view)
                vs = v_sb[kc][:, n * 384:(n + 1) * 384].copy()
                vs.ap = vs.ap[:-1] + [[64, 6], [1, 64]]
                pvr = pv[:].copy()
                pvr.ap = pvr.ap[:-1] + [[64, 6], [1, 64]]
                ebr = ebias[:, kc * H + n * 6:kc * H + n * 6 + 6].copy()
                ebr.ap = ebr.ap + [[0, 64]]
                nc.vector.tensor_mul(vs, pvr, ebr)

        for j in range(NPAIR):
            avden_pair(j, e_w0[j], ots[j])
        qt = prep_window(1)
        outproj_window(0, ots)

        # ------- windows 1..3: steady software pipeline -------
        for tc4 in range(1, NTW):
            ots = [big.tile([128, TW], BF16, tag=f"ot{j}", bufs=2, name="otw")
                   for j in range(NPAIR)]
            for j in range(NPAIR):
                e_j = scores_pair(j, qt)
                avden_pair(j, e_j, ots[j])
            if tc4 + 1 < NTW:
                qt_next = prep_window(tc4 + 1)
            else:
                qt_next = None
            outproj_window(tc4 * (TW // 128), ots)
            qt = qt_next

    nc.compile()
    return nc


def _get_program():
    if "nc" not in _CACHE:
        _CACHE["nc"] = _build_program()
    return _CACHE["nc"]


def _host_prep(inputs):
    import ml_dtypes
    bf16 = ml_dtypes.bfloat16

    f32 = lambda x: np.ascontiguousarray(np.asarray(x, dtype=np.float32))
    Wq, Wk, Wv, Wo = (f32(inputs[k]) for k in ("Wq", "Wk", "Wv", "Wo"))
    bq, bk, bv, bo = (f32(inputs[k]) for k in ("bq", "bk", "bv", "bo"))
    beta = f32(inputs["beta"])

    shared = {
        "wqt": np.ascontiguousarray((Wq.T * SCALE).astype(bf16)),
        "wkt": np.ascontiguousarray(Wk.T.astype(bf16)),
        "wvt": np.ascontiguousarray(Wv.T.astype(bf16)),
        "wot": np.ascontiguousarray(Wo.T.astype(bf16)),
        "bq": np.ascontiguousarray((bq * SCALE).reshape(DC, 128).T),
        "bk": np.ascontiguousarray(bk.reshape(DC, 128).T),
        # bv folded through Wo (sum_k softmax == 1), bo absorbed:
        "bo": np.ascontiguousarray((bo + bv @ Wo.T).reshape(1, D)),
        "beta": np.ascontiguousarray(beta.reshape(1, H)),
    }

    hs = np.asarray(inputs["hidden_states"], dtype=np.float32).astype(bf16)
    kgk = np.asarray(inputs["kg_key"], dtype=np.float32).astype(bf16)
    kgv = np.asarray(inputs["kg_value"], dtype=np.float32).astype(bf16)
    pooled = np.asarray(
        inputs["pooled_hidden_states"], dtype=np.float32).astype(bf16)

    in_maps = []
    for b in range(BS):
        m = dict(shared)
        m["hs"] = np.ascontiguousarray(hs[b])
        m["kgk"] = np.ascontiguousarray(kgk[b])
        m["kgv"] = np.ascontiguousarray(kgv[b])
        m["pooled"] = np.ascontiguousarray(pooled[b].reshape(1, D))
        in_maps.append(m)
    return in_maps




def _install_ntff_hook():
    """Register the axon NTFF profile hook so trace=True yields exec_time_ns.

    Only used from our own test harness (TRACE=True); the default kernel()
    path never calls this.
    """
    try:
        from antenv.axon_hooks import get_axon_ntff_profile_hook  # noqa: F401
        return
    except ImportError:
        pass
    import contextlib
    import ctypes
    import types

    so_path = "/opt/axon/libaxon_pjrt.so"
    try:
        lib = ctypes.CDLL(so_path)
    except OSError:
        return
    if not hasattr(lib, "axon_start_nrt_profile"):
        return
    lib.axon_start_nrt_profile.argtypes = [
        ctypes.POINTER(ctypes.c_int64), ctypes.c_size_t]
    lib.axon_start_nrt_profile.restype = ctypes.c_int64
    lib.axon_stop_nrt_profile.argtypes = [ctypes.c_char_p]
    lib.axon_stop_nrt_profile.restype = ctypes.c_int64

    @contextlib.contextmanager
    def _hook(output_dir, device_ids):
        import jax
        jax.devices()
        if device_ids:
            ids = (ctypes.c_int64 * len(device_ids))(*device_ids)
            rc = lib.axon_start_nrt_profile(ids, len(device_ids))
        else:
            rc = lib.axon_start_nrt_profile(None, 0)
        if rc != 0:
            raise RuntimeError(f"axon_start_nrt_profile rc={rc}")
        try:
            yield
        finally:
            n = lib.axon_stop_nrt_profile(str(output_dir).encode())
            print(f"profile: {n} file(s) written to {output_dir}",
                  file=sys.stderr)

    mod = types.ModuleType("antenv.axon_hooks")
    mod.get_axon_ntff_profile_hook = lambda: _hook
    mod.set_axon_ntff_profile_hook = lambda h: None
    sys.modules["antenv.axon_hooks"] = mod


def kernel(**inputs):
    global LAST_EXEC_NS
    _ensure_path()
    from concourse import bass_utils

    if TRACE:
        _install_ntff_hook()
    nc = _get_program()
    in_maps = _host_prep(inputs)
    res = bass_utils.run_bass_kernel_spmd(
        nc, in_maps, core_ids=list(range(BS)), trace=TRACE)
    LAST_EXEC_NS = res.exec_time_ns
    out = np.stack([res.results[b]["out"] for b in range(BS)], axis=0)
    return out.astype(np.float32)



# revision 44
# speedup vs baseline: 1.2048x; 1.0068x over previous
"""Trainium2 Bass kernel for nn_KnowledgeAttention.

Math (per batch example b):
    sim[k]  = cos_sim(pooled[b], kg_key[b,k])                      # [K]
    q       = (hs @ Wq.T + bq) * HD**-0.5     -> heads [T,H,HD]
    k       = kg_value @ Wk.T + bk            -> heads [K,H,HD]
    v       = kg_value @ Wv.T + bv            -> heads [K,H,HD]
    S[h,t,k]= q_h[t]·k_h[k] + beta[h]*sim[k]
    P       = softmax_k(S);  O[t,h] = sum_k P v
    out     = O @ Wo.T + bo

Sharding: pure data-parallel over batch — 8 examples on 8 cores, weights
replicated, no collectives.

Per-core strategy (contractions on the partition dim, all matmuls bf16 with
fp32 PSUM accumulation):
  - hs/kg inputs cast to bf16 on the host (halves input DMA); hs.T and
    kg_value.T via bf16 PE transposes.
  - The softmax bias exp(beta_h*sim[k]) is folded multiplicatively into v
    (during the V-projection PSUM copy) and into the denominator weights
    (stride-0 broadcast APs over ebias columns), so the exps are bias-free.
  - Scores per head pair: even head on PE row-tile (0,0), odd on (64,0) —
    the two half-array matmuls stream concurrently into one 2-bank PSUM
    tile covered by a single [128,1024] exp.
  - AV and 64x-replicated denominators: even head on col-tile (0,0) rows
    0:64, odd on (0,64) rows 64:128 — column pairs stream concurrently;
    softmax normalization is one reciprocal + one multiply per pair.
  - Software-pipelined windows: prep(w+1) (hs DMA/transpose/Q-proj) and
    out-proj(w) are issued after attention(w) so the scheduler uses them
    as PE filler during the exp-paced stretches.
  - PSUM rings: scores 2x[128,1024], AV 1, denom 1, everything else 2.
"""

import sys

import numpy as np

# ---------------------------------------------------------------- constants
BS = 8
T = 2048
D = 768
H = 12
HD = 64
K = 512
SCALE = HD ** -0.5
EPS = 1e-8
DC = D // 128   # 6 contraction/partition chunks of 128 over D
KC = K // 128   # 4 chunks over K
TW = 512        # t window for moving operand
NTW = T // TW   # 4
NPAIR = H // 2  # 6 head pairs

TRACE = False
LAST_EXEC_NS = None

_CACHE = {}


def _ensure_path():
    try:
        import concourse  # noqa: F401
    except ImportError:
        for p in ("/opt/trn_rl_repo", "/root/.axon_site/_ro/trn_rl_repo"):
            if p not in sys.path:
                sys.path.insert(0, p)


def _build_program():
    _ensure_path()
    import concourse.bass as bass
    import concourse.mybir as mybir
    import concourse.tile as tile
    from concourse import bacc
    from concourse.masks import make_identity
    from contextlib import ExitStack

    F32 = mybir.dt.float32
    BF16 = mybir.dt.bfloat16
    Alu = mybir.AluOpType
    Act = mybir.ActivationFunctionType

    nc = bacc.Bacc("TRN2", target_bir_lowering=False, debug=False, num_devices=BS)

    hs_d = nc.dram_tensor("hs", [T, D], BF16, kind="ExternalInput").ap()
    kgk_d = nc.dram_tensor("kgk", [K, D], BF16, kind="ExternalInput").ap()
    kgv_d = nc.dram_tensor("kgv", [K, D], BF16, kind="ExternalInput").ap()
    pl_d = nc.dram_tensor("pooled", [1, D], BF16, kind="ExternalInput").ap()
    wqt_d = nc.dram_tensor("wqt", [D, D], BF16, kind="ExternalInput").ap()
    wkt_d = nc.dram_tensor("wkt", [D, D], BF16, kind="ExternalInput").ap()
    wvt_d = nc.dram_tensor("wvt", [D, D], BF16, kind="ExternalInput").ap()
    wot_d = nc.dram_tensor("wot", [D, D], BF16, kind="ExternalInput").ap()
    bq_d = nc.dram_tensor("bq", [128, DC], F32, kind="ExternalInput").ap()
    bk_d = nc.dram_tensor("bk", [128, DC], F32, kind="ExternalInput").ap()
    bo_d = nc.dram_tensor("bo", [1, D], F32, kind="ExternalInput").ap()
    beta_d = nc.dram_tensor("beta", [1, H], F32, kind="ExternalInput").ap()
    out_d = nc.dram_tensor("out", [T, D], F32, kind="ExternalOutput").ap()

    with tile.TileContext(nc) as tc, ExitStack() as ctx:
        const = ctx.enter_context(tc.tile_pool(name="const", bufs=1))
        inp = ctx.enter_context(tc.tile_pool(name="inp", bufs=6))
        wpool = ctx.enter_context(tc.tile_pool(name="w", bufs=12))
        big = ctx.enter_context(tc.tile_pool(name="big", bufs=12))
        hstw_p = ctx.enter_context(tc.tile_pool(name="hstw", bufs=12))
        kt_p = ctx.enter_context(tc.tile_pool(name="ktp", bufs=6))
        v_p = ctx.enter_context(tc.tile_pool(name="vp", bufs=4))
        kgt_p = ctx.enter_context(tc.tile_pool(name="kgtp", bufs=6))
        e_p = ctx.enter_context(tc.tile_pool(name="ep", bufs=12))
        r_p = ctx.enter_context(tc.tile_pool(name="rp", bufs=4))
        rb_p = ctx.enter_context(tc.tile_pool(name="rbp", bufs=2))
        fin_p = ctx.enter_context(tc.tile_pool(name="finp", bufs=2))
        sm_p = ctx.enter_context(tc.tile_pool(name="smp", bufs=4))
        ps = ctx.enter_context(tc.tile_pool(name="ps", bufs=2, space="PSUM"))

        # ---------------- phase 0: constants + cosine-sim bias ----------------
        ident = const.tile([128, 128], BF16, tag="ident")
        make_identity(nc, ident[:])
        ones_bf = const.tile([128, 64], BF16, tag="ones_bf")
        nc.vector.memset(ones_bf[:], 1.0)
        # kg_value loads first: transposes are the critical path into phase 1
        kv_tiles = []
        for c in range(KC):
            kv = inp.tile([128, D], BF16, tag="inp", name="kv")
            nc.sync.dma_start(kv[:], kgv_d[c * 128:(c + 1) * 128, :])
            kv_tiles.append(kv)
        wk_sb = []
        for c in range(DC):
            wk = wpool.tile([128, D], BF16, tag="w")
            nc.sync.dma_start(wk[:], wkt_d[c * 128:(c + 1) * 128, :])
            wk_sb.append(wk)
        kk_tiles = []
        for c in range(KC):
            kk = inp.tile([128, D], BF16, tag="kgk", bufs=4, name="kk")
            nc.sync.dma_start(kk[:], kgk_d[c * 128:(c + 1) * 128, :])
            kk_tiles.append(kk)

        pl = const.tile([1, D], BF16, tag="pl")
        nc.sync.dma_start(pl[:], pl_d)
        bt = const.tile([1, H], F32, tag="bt")
        nc.sync.dma_start(bt[:], beta_d)
        bo_row = const.tile([1, D], F32, tag="bo_row")
        nc.sync.dma_start(bo_row[:], bo_d)
        bq_sb = const.tile([128, DC], F32, tag="bq_sb")
        nc.sync.dma_start(bq_sb[:], bq_d)
        bk_sb = const.tile([128, DC], F32, tag="bk_sb")
        nc.sync.dma_start(bk_sb[:], bk_d)

        bo_bc = const.tile([128, D], F32, tag="bo_bc")
        nc.gpsimd.partition_broadcast(bo_bc[:], bo_row[:], channels=128)
        beta_bc = const.tile([128, H], F32, tag="beta_bc")
        nc.gpsimd.partition_broadcast(beta_bc[:], bt[:], channels=128)
        pl_bc = const.tile([128, D], BF16, tag="pl_bc")
        nc.gpsimd.partition_broadcast(pl_bc[:], pl[:], channels=128)

        # pooled 1/||.|| as a per-partition vector (computed on the broadcast)
        pl_sq = inp.tile([128, D], BF16, tag="inp", name="pl_sq")
        pnorm = sm_p.tile([128, 1], F32, tag="pnorm")
        nc.scalar.activation(pl_sq[:], pl_bc[:], Act.Square, accum_out=pnorm[:])
        nc.scalar.activation(pnorm[:], pnorm[:], Act.Sqrt)
        nc.vector.tensor_scalar_max(pnorm[:], pnorm[:], EPS)
        rp_vec = const.tile([128, 1], F32, tag="rp_vec")
        nc.vector.reciprocal(rp_vec[:], pnorm[:])

        # ---------------- phase 1a: kg_value.T, k.T ----------------
        # remaining weight DMAs in consumption order: wq (prep0), wv, wo
        wq_sb = []
        for c in range(DC):
            wq = wpool.tile([128, D], BF16, tag="w")
            nc.sync.dma_start(wq[:], wqt_d[c * 128:(c + 1) * 128, :])
            wq_sb.append(wq)
        hv0_tiles = []
        for tsub in range(TW // 128):
            hv = inp.tile([128, D], BF16, tag="inp", name="hv")
            nc.sync.dma_start(hv[:], hs_d[tsub * 128:(tsub + 1) * 128, :])
            hv0_tiles.append(hv)

        kgt = [kgt_p.tile([128, K], BF16, tag="kgt", name="kgt") for _ in range(DC)]
        for dchunk in range(DC):
            pt = ps.tile([128, K], BF16, tag="mm", bufs=2, name="ptr")
            for c in range(KC):
                nc.tensor.transpose(
                    pt[:, c * 128:(c + 1) * 128],
                    kv_tiles[c][:, dchunk * 128:(dchunk + 1) * 128], ident[:])
            nc.vector.tensor_copy(kgt[dchunk][:], pt[:])

        kt = [kt_p.tile([128, K], BF16, tag="kt", name="kt") for _ in range(DC)]
        for m in range(DC):
            pk = ps.tile([128, K], F32, tag="mm", bufs=2)
            for c in range(DC):
                nc.tensor.matmul(
                    pk[:], wk_sb[c][:, m * 128:(m + 1) * 128], kgt[c][:],
                    start=(c == 0), stop=(c == DC - 1))
            nc.vector.tensor_scalar_add(kt[m][:], pk[:], bk_sb[:, m:m + 1])

        def prep_window(tc4, hv_tiles=None):
            hstw = [hstw_p.tile([128, TW], BF16, tag="hstw", name="hstw")
                    for _ in range(DC)]
            if hv_tiles is None:
                hv_tiles = []
                for tsub in range(TW // 128):
                    hv = inp.tile([128, D], BF16, tag="inp", name="hv")
                    t0 = tc4 * TW + tsub * 128
                    nc.sync.dma_start(hv[:], hs_d[t0:t0 + 128, :])
                    hv_tiles.append(hv)
            for c in range(DC):
                pt = ps.tile([128, TW], BF16, tag="mm", bufs=2, name="ptr")
                for tsub in range(TW // 128):
                    nc.tensor.transpose(
                        pt[:, tsub * 128:(tsub + 1) * 128],
                        hv_tiles[tsub][:, c * 128:(c + 1) * 128], ident[:])
                nc.vector.tensor_copy(hstw[c][:], pt[:])
            qts = [big.tile([128, TW], BF16, tag=f"qt{m}", bufs=2, name="qtw")
                   for m in range(DC)]
            for m in range(DC):
                pq = ps.tile([128, TW], F32, tag="mm", bufs=2)
                for c in range(DC):
                    nc.tensor.matmul(
                        pq[:], wq_sb[c][:, m * 128:(m + 1) * 128], hstw[c][:],
                        start=(c == 0), stop=(c == DC - 1))
                nc.vector.tensor_scalar_add(qts[m][:], pq[:], bq_sb[:, m:m + 1])
            return qts

        qt = prep_window(0, hv0_tiles)

        # ------- cosine-sim bias + ebias (ACT/DVE, overlaps PE prep) -------
        bias_all = const.tile([128, KC * H], F32, tag="bias_all")
        for c in range(KC):
            kk = kk_tiles[c]
            sq = inp.tile([128, D], BF16, tag="inp")
            nrm = sm_p.tile([128, 1], F32, tag="nrm")
            nc.scalar.activation(sq[:], kk[:], Act.Square, accum_out=nrm[:])
            nc.scalar.activation(nrm[:], nrm[:], Act.Sqrt)
            nc.vector.tensor_scalar_max(nrm[:], nrm[:], EPS)
            rn = sm_p.tile([128, 1], F32, tag="rn")
            nc.vector.reciprocal(rn[:], nrm[:])
            sq2 = inp.tile([128, D], BF16, tag="inp")
            dot = sm_p.tile([128, 1], F32, tag="dot")
            nc.vector.scalar_tensor_tensor(
                out=sq2[:], in0=kk[:], scalar=1.0, in1=pl_bc[:],
                op0=Alu.mult, op1=Alu.mult, accum_out=dot[:])
            nc.vector.tensor_mul(dot[:], dot[:], rn[:])
            nc.vector.tensor_mul(dot[:], dot[:], rp_vec[:])
            nc.vector.tensor_scalar_mul(
                bias_all[:, c * H:(c + 1) * H], beta_bc[:], dot[:])

        # ebias[k_part, kc*H + h] = exp(beta[h]*sim[k]) — folded into v and
        # the denominator weights so the softmax exp needs no bias (enables
        # even/odd-merged [128,1024] exps)
        ebias = const.tile([128, KC * H], F32, tag="ebias")
        nc.scalar.activation(ebias[:], bias_all[:], Act.Exp)
        ebias_bf = const.tile([128, KC * H], BF16, tag="ebias_bf")
        nc.vector.tensor_copy(ebias_bf[:], ebias[:])

        def eb_col64(c, h):
            # [128, 64] broadcast view of ebias_bf column kc*H+h (stride-0)
            col = ebias_bf[:, c * H + h:c * H + h + 1].copy()
            col.ap = col.ap[:-1] + [[0, 64]]
            return col

        # scores: even head on PE row-tile (0,0) -> cols 0:512 of a 2-bank
        # tile, odd head on (64,0) -> cols 512:1024; one bias-free exp
        # covers both banks.
        def scores_pair(j, qtw, ts0=0, w=TW):
            # odd head always lands at the TW (bank-aligned) offset; for
            # w < TW one exp covers [0, TW+w) and cols [w, TW) are unused.
            e_j = []
            for kc in range(KC):
                psc = ps.tile([128, TW + w], F32, tag="s", bufs=2)
                nc.tensor.matmul(
                    psc[:, 0:w], kt[j][0:64, kc * 128:(kc + 1) * 128],
                    qtw[j][0:64, ts0:ts0 + w], start=True, stop=True)
                nc.tensor.matmul(
                    psc[:, TW:TW + w],
                    kt[j][64:128, kc * 128:(kc + 1) * 128],
                    qtw[j][64:128, ts0:ts0 + w], start=True, stop=True)
                ebig = e_p.tile([128, 2 * w], BF16, tag="e", bufs=24)
                src = psc[:].copy()
                src.ap = src.ap[:-1] + [[TW, 2], [1, w]]
                dst = ebig[:].copy()
                dst.ap = dst.ap[:-1] + [[w, 2], [1, w]]
                nc.scalar.activation(dst, src, Act.Exp)
                e_j.append(ebig)
            return e_j

        # AV + denominators: even head -> col-tile (0,0) rows 0:64, odd head
        # -> col-tile (0,64) rows 64:128 of the same bank; interleaved issue
        # so both column tiles stream concurrently. pd rows 0:64 = sum_k
        # e_even (64x replicated), rows 64:128 = odd.
        def avden_pair(j, e_j, ot_j, w=TW):
            po = ps.tile([128, w], F32, tag="o", bufs=1, name="po")
            pd = ps.tile([128, w], F32, tag="d", bufs=1, name="pd")
            for kc in range(KC):
                st = (kc == 0)
                sp = (kc == KC - 1)
                nc.tensor.matmul(
                    po[0:64, :],
                    v_sb[kc][:, (2 * j) * HD:(2 * j + 1) * HD],
                    e_j[kc][:, 0:w], start=st, stop=sp)
                nc.tensor.matmul(
                    po[64:128, :],
                    v_sb[kc][:, (2 * j + 1) * HD:(2 * j + 2) * HD],
                    e_j[kc][:, w:2 * w], start=st, stop=sp)
                nc.tensor.matmul(
                    pd[0:64, :], eb_col64(kc, 2 * j),
                    e_j[kc][:, 0:w], start=st, stop=sp)
                nc.tensor.matmul(
                    pd[64:128, :], eb_col64(kc, 2 * j + 1),
                    e_j[kc][:, w:2 * w], start=st, stop=sp)
            rall = r_p.tile([128, w], F32, tag="rall", name="rall")
            # high priority: these free the po/pd banks — keep them ahead
            # of filler DVE work so the next pair's AV/den can start
            with tc.high_priority():
                nc.vector.reciprocal_approx_fast(rall[:], pd[:])
                nc.vector.tensor_mul(ot_j[:], po[:], rall[:])

        def outproj_window(tc16_0, ots, w=TW):
            for tsub in range(w // 128):
                tc16 = tc16_0 + tsub
                fin = fin_p.tile([128, D], F32, tag="fin")
                for n in range(2):
                    pf = ps.tile([128, 384], F32, tag="mm", bufs=2)
                    for c in range(DC):
                        nc.tensor.matmul(
                            pf[:], ots[c][:, tsub * 128:(tsub + 1) * 128],
                            wo_sb[c][:, n * 384:(n + 1) * 384],
                            start=(c == 0), stop=(c == DC - 1))
                    nc.vector.tensor_add(
                        fin[:, n * 384:(n + 1) * 384], pf[:],
                        bo_bc[:, n * 384:(n + 1) * 384])
                    # store each half as soon as its bias-add lands
                    nc.sync.dma_start(
                        out_d[tc16 * 128:(tc16 + 1) * 128,
                              n * 384:(n + 1) * 384],
                        fin[:, n * 384:(n + 1) * 384])

        # ------- window 0: all scores issued first; the v projection below
        # is the PE filler for the exp-paced stretch; AV/den then consume.
        ots = [big.tile([128, TW], BF16, tag=f"ot{j}", bufs=2, name="otw")
               for j in range(NPAIR)]
        e_w0 = [scores_pair(j, qt) for j in range(NPAIR)]

        # ---------------- v projection (+ ebias fold) ----------------
        wv_sb = []
        for c in range(DC):
            wv = wpool.tile([128, D], BF16, tag="w")
            nc.sync.dma_start(wv[:], wvt_d[c * 128:(c + 1) * 128, :])
            wv_sb.append(wv)
        wo_sb = []
        for c in range(DC):
            wo = wpool.tile([128, D], BF16, tag="w")
            nc.sync.dma_start(wo[:], wot_d[c * 128:(c + 1) * 128, :])
            wo_sb.append(wo)

        v_sb = [v_p.tile([128, D], BF16, tag="v", name="vsb")
                for _ in range(KC)]
        for kc in range(KC):
            for n in range(2):
                pv = ps.tile([128, 384], F32, tag="mm", bufs=2)
                for c in range(DC):
                    nc.tensor.matmul(
                        pv[:], kgt[c][:, kc * 128:(kc + 1) * 128],
                        wv_sb[c][:, n * 384:(n + 1) * 384],
                        start=(c == 0), stop=(c == DC - 1))
                # copy out with the per-head exp(beta*sim) factor folded in:
                # one broadcast multiply over all 6 heads ([128, 6, 64] view)
                vs = v_sb[kc][:, n * 384:(n + 1) * 384].copy()
                vs.ap = vs.ap[:-1] + [[64, 6], [1, 64]]
                pvr = pv[:].copy()
                pvr.ap = pvr.ap[:-1] + [[64, 6], [1, 64]]
                ebr = ebias[:, kc * H + n * 6:kc * H + n * 6 + 6].copy()
                ebr.ap = ebr.ap + [[0, 64]]
                nc.vector.tensor_mul(vs, pvr, ebr)

        for j in range(NPAIR):
            avden_pair(j, e_w0[j], ots[j])
        qt = prep_window(1)
        outproj_window(0, ots)

        # ------- windows 1..3: steady software pipeline -------
        for tc4 in range(1, NTW):
            ots = [big.tile([128, TW], BF16, tag=f"ot{j}", bufs=2, name="otw")
                   for j in range(NPAIR)]
            for j in range(NPAIR):
                e_j = scores_pair(j, qt)
                avden_pair(j, e_j, ots[j])
            if tc4 + 1 < NTW:
                qt_next = prep_window(tc4 + 1)
            else:
                qt_next = None
            outproj_window(tc4 * (TW // 128), ots)
            qt = qt_next

    nc.compile()
    return nc


def _get_program():
    if "nc" not in _CACHE:
        _CACHE["nc"] = _build_program()
    return _CACHE["nc"]


def _host_prep(inputs):
    import ml_dtypes
    bf16 = ml_dtypes.bfloat16

    f32 = lambda x: np.ascontiguousarray(np.asarray(x, dtype=np.float32))
    Wq, Wk, Wv, Wo = (f32(inputs[k]) for k in ("Wq", "Wk", "Wv", "Wo"))
    bq, bk, bv, bo = (f32(inputs[k]) for k in ("bq", "bk", "bv", "bo"))
    beta = f32(inputs["beta"])

    shared = {
        "wqt": np.ascontiguousarray((Wq.T * SCALE).astype(bf16)),
        "wkt": np.ascontiguousarray(Wk.T.astype(bf16)),
        "wvt": np.ascontiguousarray(Wv.T.astype(bf16)),
        "wot": np.ascontiguousarray(Wo.T.astype(bf16)),
        "bq": np.ascontiguousarray((bq * SCALE).reshape(DC, 128).T),
        "bk": np.ascontiguousarray(bk.reshape(DC, 128).T),
        # bv folded through Wo (sum_k softmax == 1), bo absorbed:
        "bo": np.ascontiguousarray((bo + bv @ Wo.T).reshape(1, D)),
        "beta": np.ascontiguousarray(beta.reshape(1, H)),
    }

    hs = np.asarray(inputs["hidden_states"], dtype=np.float32).astype(bf16)
    kgk = np.asarray(inputs["kg_key"], dtype=np.float32).astype(bf16)
    kgv = np.asarray(inputs["kg_value"], dtype=np.float32).astype(bf16)
    pooled = np.asarray(
        inputs["pooled_hidden_states"], dtype=np.float32).astype(bf16)

    in_maps = []
    for b in range(BS):
        m = dict(shared)
        m["hs"] = np.ascontiguousarray(hs[b])
        m["kgk"] = np.ascontiguousarray(kgk[b])
        m["kgv"] = np.ascontiguousarray(kgv[b])
        m["pooled"] = np.ascontiguousarray(pooled[b].reshape(1, D))
        in_maps.append(m)
    return in_maps




def _install_ntff_hook():
    """Register the axon NTFF profile hook so trace=True yields exec_time_ns.

    Only used from our own test harness (TRACE=True); the default kernel()
    path never calls this.
    """
    try:
        from antenv.axon_hooks import get_axon_ntff_profile_hook  # noqa: F401
        return
    except ImportError:
        pass
    import contextlib
    import ctypes
    import types

    so_path = "/opt/axon/libaxon_pjrt.so"
    try:
        lib = ctypes.CDLL(so_path)
    except OSError:
        return
    if not hasattr(lib, "axon_start_nrt_profile"):
        return
    lib.axon_start_nrt_profile.argtypes = [
        ctypes.POINTER(ctypes.c_int64), ctypes.c_size_t]
    lib.axon_start_nrt_profile.restype = ctypes.c_int64
    lib.axon_stop_nrt_profile.argtypes = [ctypes.c_char_p]
    lib.axon_stop_nrt_profile.restype = ctypes.c_int64

    @contextlib.contextmanager
    def _hook(output_dir, device_ids):
        import jax
        jax.devices()
        if device_ids:
            ids = (ctypes.c_int64 * len(device_ids))(*device_ids)
            rc = lib.axon_start_nrt_profile(ids, len(device_ids))
        else:
            rc = lib.axon_start_nrt_profile(None, 0)
        if rc != 0:
            raise RuntimeError(f"axon_start_nrt_profile rc={rc}")
        try:
            yield
        finally:
            n = lib.axon_stop_nrt_profile(str(output_dir).encode())
            print(f"profile: {n} file(s) written to {output_dir}",
                  file=sys.stderr)

    mod = types.ModuleType("antenv.axon_hooks")
    mod.get_axon_ntff_profile_hook = lambda: _hook
    mod.set_axon_ntff_profile_hook = lambda h: None
    sys.modules["antenv.axon_hooks"] = mod


def kernel(**inputs):
    global LAST_EXEC_NS
    _ensure_path()
    from concourse import bass_utils

    if TRACE:
        _install_ntff_hook()
    nc = _get_program()
    in_maps = _host_prep(inputs)
    res = bass_utils.run_bass_kernel_spmd(
        nc, in_maps, core_ids=list(range(BS)), trace=TRACE)
    LAST_EXEC_NS = res.exec_time_ns
    out = np.stack([res.results[b]["out"] for b in range(BS)], axis=0)
    return out.astype(np.float32)

